# revision 14
# baseline (speedup 1.0000x reference)
"""Trainium2 Bass kernel: nn_ConditionalContrastiveLoss, SPMD across 8 NeuronCores.

Strategy (data parallel over rows, per sharding hint):
  - Host sorts rows by label (loss is row-permutation invariant). Each core
    owns 1024 rows and computes its 1024 x 8192 slice of the similarity
    matrix against the full embedding set (columns), which the host hands to
    every core in transposed bf16 layout, column-rotated so that the core's
    own rows sit at a fixed column offset M. With sorted labels, all
    positive pairs (same label) of a 128-row block then live in a fixed
    +-M column window around the diagonal -> one fused DVE op per block
    extracts the masked positive sum; a second extracts the diagonal.
  - Row normalization happens on device: column norms via ones-matmul over
    the squared transposed matrix, rsqrt, broadcast, elementwise scale.
  - exp(2*cos) row sums ride the ScalarEngine's fused accumulate while it
    reads 2048-wide PSUM chunks produced by bf16 matmuls.
  - Each core reduces its rows' -log(num/den) to one scalar; host sums the
    8 partials (the "all-reduce") and divides by N.
"""
import numpy as np
import ml_dtypes

from concourse import bacc, mybir
from concourse import tile
from concourse.bass_utils import run_bass_kernel_spmd

N, D, NCORES = 8192, 128, 8
NL = N // NCORES          # rows per core
RB = NL // 128            # 128-row blocks per core
CQ = 2048                 # PSUM/ACT chunk width
NCQ = N // CQ
BF16 = mybir.dt.bfloat16
F32 = mybir.dt.float32
I32 = mybir.dt.int32
AX = mybir.AxisListType
OP = mybir.AluOpType
AF = mybir.ActivationFunctionType

_cache: dict = {}


def _build(M: int):
    W = 128 + 2 * M
    LABW = 1024 + 2 * M
    assert M + NL + 128 <= CQ and LABW <= CQ

    nc = bacc.Bacc("TRN2", target_bir_lowering=False, debug=False,
                   num_devices=NCORES)
    at_d = nc.declare_dram_parameter("at", [D, N], BF16, isOutput=False)
    lab_d = nc.declare_dram_parameter("lab", [128, LABW], F32, isOutput=False)
    iota_d = nc.declare_dram_parameter("iotaw", [128, W], F32, isOutput=False)
    labr_d = nc.declare_dram_parameter("labr", [128, RB], F32, isOutput=False)
    er_d = nc.declare_dram_parameter("erows", [NL, D], F32, isOutput=False)
    pr_d = nc.declare_dram_parameter("prows", [NL, D], F32, isOutput=False)
    out_d = nc.declare_dram_parameter("out", [1, 1], F32, isOutput=True)
    dbg_d = nc.declare_dram_parameter("dbg", [128, 6 * RB], F32, isOutput=True)

    with tile.TileContext(nc) as tc:
        with tc.tile_pool(name="persist", bufs=1) as pp, \
             tc.tile_pool(name="work", bufs=3) as wp, \
             tc.tile_pool(name="psum", bufs=2, space="PSUM") as pm:
            atc = [pp.tile([D, CQ], BF16, name=f"atn{k}", tag=f"atn{k}")
                   for k in range(NCQ)]
            lab_bc = pp.tile([128, LABW], F32, tag="lab_bc")
            labr = pp.tile([128, RB], F32, tag="labr")
            iota_f = pp.tile([128, W], F32, tag="iota_f")
            ones16 = pp.tile([128, 1], BF16, tag="ones16")
            ones32 = pp.tile([128, 1], F32, tag="ones32")
            ones_row = pp.tile([1, 128], F32, tag="ones_row")
            at_sb = pp.tile([D, N], BF16, tag="at_sb")
    
            nst_row = pp.tile([1, N], F32, tag="nst_row")
            r_row = pp.tile([1, N], F32, tag="r_row")
            st = {k: pp.tile([128, RB], F32, name="st_" + k, tag="st_" + k)
                  for k in ("rs", "pos", "diag", "ne", "npx", "dot")}

            nc.vector.memset(ones16[:], 1.0)
            nc.vector.memset(ones32[:], 1.0)
            nc.vector.memset(ones_row[:], 1.0)
            nc.sync.dma_start(iota_f[:], iota_d[:])
            nc.sync.dma_start(lab_bc[:], lab_d[:])
            nc.sync.dma_start(labr[:], labr_d[:])

            # ---- stage B: load + column-normalize, pipelined per chunk ----
            for k in range(N // 1024):
                sl = slice(k * 1024, (k + 1) * 1024)
                nc.sync.dma_start(at_sb[:, sl], at_d[:, sl])
            for k in range(NCQ):
                sl = slice(k * CQ, (k + 1) * CQ)
                sqc = wp.tile([128, CQ], BF16, name="sqc", tag="sq")
                for q in range(CQ // 512):
                    q2 = slice(k * CQ + q * 512, k * CQ + (q + 1) * 512)
                    nc.vector.tensor_tensor(sqc[:, q * 512:(q + 1) * 512],
                                            at_sb[:, q2], at_sb[:, q2],
                                            op=OP.mult)
                nsq = pm.tile([1, CQ], F32, name="nsq", tag="g")
                for q in range(CQ // 512):
                    qs = slice(q * 512, (q + 1) * 512)
                    gs = slice(k * CQ + q * 512, k * CQ + (q + 1) * 512)
                    nc.tensor.matmul(nsq[:, qs], ones16[:], sqc[:, qs],
                                     start=True, stop=True)
                    nc.scalar.activation(nst_row[0:1, gs], nsq[:, qs], AF.Sqrt)
                    nc.vector.reciprocal(r_row[0:1, gs], nst_row[0:1, gs])
                    rbc = pm.tile([128, 512], F32, name="rbc", tag="g")
                    nc.tensor.matmul(rbc[:], ones_row[:], r_row[0:1, gs],
                                     start=True, stop=True)
                    nc.vector.tensor_tensor(atc[k][:, qs], at_sb[:, gs],
                                            rbc[:], op=OP.mult)

            # ---- stage C: sim row blocks; exp+rowsum; window pos/diag ----
            for rb in range(RB):
                lh = atc[0][:, M + rb * 128: M + rb * 128 + 128]
                rsp = wp.tile([128, NCQ], F32, name="rsp", tag="rsp")
                e0 = wp.tile([128, CQ], BF16, name="e0", tag="e0")
                for cq in range(NCQ):
                    g = pm.tile([128, CQ], F32, name="g", tag="g")
                    for q in range(CQ // 512):
                        qs = slice(q * 512, (q + 1) * 512)
                        nc.tensor.matmul(g[:, qs], lh, atc[cq][:, qs],
                                         start=True, stop=True)
                    eout = e0 if cq == 0 else wp.tile([128, CQ], BF16,
                                                      name="escr", tag="escr")
                    nc.scalar.activation(eout[:], g[:], AF.Exp, scale=2.0,
                                         accum_out=rsp[:, cq:cq + 1])
                nc.vector.reduce_sum(st["rs"][:, rb:rb + 1], rsp[:],
                                     axis=AX.X)
                so = rb * 128
                stt1 = wp.tile([128, W], F32, name="stt1", tag="stt1")
                stt2 = wp.tile([128, W], F32, name="stt2", tag="stt2")
                nc.vector.scalar_tensor_tensor(
                    stt1[:], lab_bc[:, so:so + W], labr[:, rb:rb + 1],
                    e0[:, so:so + W], OP.is_equal, OP.mult,
                    accum_out=st["pos"][:, rb:rb + 1])
                nc.vector.scalar_tensor_tensor(
                    stt2[:], iota_f[:], float(M), e0[:, so:so + W],
                    OP.is_equal, OP.mult,
                    accum_out=st["diag"][:, rb:rb + 1])

            # ---- stage D: embed-to-proxy ----
            for rb in range(RB):
                rsl = slice(rb * 128, (rb + 1) * 128)
                er_t = wp.tile([128, D], F32, name="er_t", tag="er")
                pr_t = wp.tile([128, D], F32, name="pr_t", tag="pr")
                nc.sync.dma_start(er_t[:], er_d[rsl, :])
                nc.sync.dma_start(pr_t[:], pr_d[rsl, :])
                s1 = wp.tile([128, D], F32, name="s1", tag="s1")
                s2 = wp.tile([128, D], F32, name="s2", tag="s2")
                s3 = wp.tile([128, D], F32, name="s3", tag="s3")
                nc.vector.scalar_tensor_tensor(
                    s1[:], er_t[:], 0.0, er_t[:], OP.bypass, OP.mult,
                    accum_out=st["ne"][:, rb:rb + 1])
                nc.vector.scalar_tensor_tensor(
                    s2[:], pr_t[:], 0.0, pr_t[:], OP.bypass, OP.mult,
                    accum_out=st["npx"][:, rb:rb + 1])
                nc.vector.scalar_tensor_tensor(
                    s3[:], er_t[:], 0.0, pr_t[:], OP.bypass, OP.mult,
                    accum_out=st["dot"][:, rb:rb + 1])

            # ---- stage E: assemble per-row loss, reduce ----
            names = ("sq_e", "sq_p", "rne", "rnp", "sc", "arg", "e2p",
                     "num1", "num2", "den1", "den2", "rden", "ratio", "lt")
            t = {n: pp.tile([128, RB], F32, name=n, tag=n) for n in names}
            lts = pp.tile([128, 1], F32, tag="lts")
            outsb = pp.tile([1, 1], F32, tag="outsb")

            nc.scalar.activation(t["sq_e"][:], st["ne"][:], AF.Sqrt)
            nc.vector.reciprocal(t["rne"][:], t["sq_e"][:])
            nc.scalar.activation(t["sq_p"][:], st["npx"][:], AF.Sqrt)
            nc.vector.reciprocal(t["rnp"][:], t["sq_p"][:])
            nc.vector.tensor_tensor(t["sc"][:], t["rne"][:], t["rnp"][:],
                                    op=OP.mult)
            nc.vector.tensor_tensor(t["arg"][:], t["sc"][:], st["dot"][:],
                                    op=OP.mult)
            nc.scalar.activation(t["e2p"][:], t["arg"][:], AF.Exp, scale=2.0)
            nc.vector.tensor_tensor(t["num1"][:], st["pos"][:], st["diag"][:],
                                    op=OP.subtract)
            nc.vector.tensor_tensor(t["num2"][:], t["num1"][:], t["e2p"][:],
                                    op=OP.add)
            nc.vector.tensor_tensor(t["den1"][:], st["rs"][:], st["diag"][:],
                                    op=OP.subtract)
            nc.vector.tensor_tensor(t["den2"][:], t["den1"][:], t["e2p"][:],
                                    op=OP.add)
            nc.vector.reciprocal(t["rden"][:], t["den2"][:])
            nc.vector.tensor_tensor(t["ratio"][:], t["num2"][:], t["rden"][:],
                                    op=OP.mult)
            nc.scalar.activation(t["lt"][:], t["ratio"][:], AF.Ln)
            nc.vector.reduce_sum(lts[:], t["lt"][:], axis=AX.X)
            ps11 = pm.tile([1, 1], F32, name="ps11", tag="g")
            nc.tensor.matmul(ps11[:], lts[:], ones32[:], start=True, stop=True)
            nc.scalar.copy(outsb[:], ps11[:])
            nc.sync.dma_start(out_d[0:1, :], outsb[:])
            for i, (k2, t2) in enumerate(
                    (("rs", st["rs"]), ("pos", st["pos"]),
                     ("diag", st["diag"]), ("e2p", t["e2p"]),
                     ("num", t["num2"]), ("den", t["den2"]))):
                nc.sync.dma_start(dbg_d[:, i * RB:(i + 1) * RB], t2[:])

    nc.finalize()
    return nc


def _prep_inputs(embed, proxy, label):
    embed = np.asarray(embed, dtype=np.float32)
    proxy = np.asarray(proxy, dtype=np.float32)
    lab = np.asarray(label)
    perm = np.argsort(lab, kind="stable")
    slab = lab[perm]
    semb = embed[perm]
    sprox = proxy[perm]

    il = slab.astype(np.int64)
    starts = np.searchsorted(il, il, side="left")
    ends = np.searchsorted(il, il, side="right")
    b0 = (np.arange(N) // 128) * 128
    m_req = max(int(np.max(b0 - starts)), int(np.max(ends - (b0 + 128))), 0)
    M = int(max(128, 64 * int(np.ceil(m_req / 64.0))))
    LABW = 1024 + 2 * M

    atT = np.ascontiguousarray(semb.T).astype(ml_dtypes.bfloat16)
    labf = slab.astype(np.float32)
    W = 128 + 2 * M
    iotaw = np.ascontiguousarray(
        (np.arange(W)[None, :] - np.arange(128)[:, None]).astype(np.float32))
    in_maps = []
    for c in range(NCORES):
        shift = M - c * NL
        at_c = np.ascontiguousarray(np.roll(atT, shift, axis=1))
        lab_c = np.ascontiguousarray(
            np.broadcast_to(np.roll(labf, shift)[:LABW], (128, LABW)))
        labr_c = np.ascontiguousarray(
            labf[c * NL:(c + 1) * NL].reshape(RB, 128).T)
        er_c = np.ascontiguousarray(semb[c * NL:(c + 1) * NL])
        pr_c = np.ascontiguousarray(sprox[c * NL:(c + 1) * NL])
        in_maps.append({"at": at_c, "lab": lab_c, "labr": labr_c,
                        "erows": er_c, "prows": pr_c, "iotaw": iotaw})
    return M, in_maps


def kernel(embed, proxy, label):
    M, in_maps = _prep_inputs(embed, proxy, label)
    nc = _cache.get(M)
    if nc is None:
        nc = _build(M)
        _cache[M] = nc
    res = run_bass_kernel_spmd(nc, in_maps, core_ids=list(range(NCORES)))
    total = sum(float(res.results[c]["out"][0, 0]) for c in range(NCORES))
    return np.array(-total / N, dtype=np.float32)
